# revision 8
# baseline (speedup 1.0000x reference)
"""Trainium2 Bass kernel for nn_Conjunction_57793079935283.

Math: the reference expands weights via ROW_IDX = tile(arange(16), 32)
(only weight rows 0..15 are used; feature i belongs to group g = i//16
with weight row r = i%16).  The whole computation collapses to

  m[b,r] = max_g |x[b, 16g+r]|
  s[b,r] = sum_g |x[b, 16g+r]|
  A[b,r] = sum_g relu(x[b,16g+r] + 1)
  B'[b,r] = sum_g sign(x[b,16g+r] + 1)          (mask count B = (B'+32)/2)

  out = A@w16 - 0.5*B'@w16 - 16*colsum(w16)     ( == (x*mask) @ W )
        - 0.1 * (s @ |w16|)                     ( == -0.1*sum-part )
        + max_r m[b,r] * (0.1*|w16[r,o]|)       ( == +0.1*max-part )

Sharding: tensor-parallel over out_features (8 cores x 128 columns).
Per core the max-part multiply m[b,r]*0.1|w[r,o]| is done on the Tensor
engine as a matmul against a block-diagonal rhs D (host-precomputed from
weights), giving tmp[b, (r,o)] in PSUM; a single strided reduce_max over
r yields the max-part.  The matmul part runs as one K=49 matmul that
accumulates on top of the reduce_max result already sitting in PSUM.
"""

import numpy as np

_PROG = None

B = 128          # batch
G = 32           # groups per feature row
R = 16           # weight rows used (multiplicity)
OUT = 1024       # out features
NCORES = 8
OC = OUT // NCORES  # out cols per core (128)


def _build_program():
    import concourse.bacc as bacc
    import concourse.mybir as mybir
    import concourse.tile as tile

    nc = bacc.Bacc(
        "TRN2", target_bir_lowering=False, debug=False, enable_asserts=False
    )
    f32 = mybir.dt.float32
    bf16 = mybir.dt.bfloat16
    AX = mybir.AxisListType
    Alu = mybir.AluOpType
    Act = mybir.ActivationFunctionType

    x_d = nc.dram_tensor("x", [B, G * R], f32, kind="ExternalInput")
    d_d = nc.dram_tensor("d", [R, R * OC], bf16, kind="ExternalInput")
    rhs_d = nc.dram_tensor("rhs", [3 * R + 1, OC], f32, kind="ExternalInput")
    out_d = nc.dram_tensor("out", [B, OC], f32, kind="ExternalOutput")
    ident_d = nc.inline_tensor(np.eye(B, dtype=np.float32), "ident")

    with tile.TileContext(nc) as tc:
        with (
            tc.tile_pool(name="sb", bufs=1) as sb,
            tc.tile_pool(name="ps", bufs=1, space="PSUM") as ps,
        ):
            x = sb.tile([B, G * R], f32)
            d = sb.tile([R, R * OC], bf16)
            rhs = sb.tile([3 * R + 1, OC], f32)
            ident = sb.tile([B, B], f32)
            nc.sync.dma_start(x[:, 0:256], x_d[:, 0:256])
            nc.sync.dma_start(x[:, 256:512], x_d[:, 256:512])
            nc.sync.dma_start(d[:], d_d[:])
            nc.sync.dma_start(rhs[:], rhs_d[:])
            nc.sync.dma_start(ident[:], ident_d[:])

            m = sb.tile([B, R], f32)
            stack3 = sb.tile([B, 3 * R + 1], f32)
            relu1 = sb.tile([B, G * R], f32)
            sgn1 = sb.tile([B, G * R], f32)
            lhsT = sb.tile([3 * R + 1, B], f32)
            mT = sb.tile([R, B], bf16)

            xg = x[:].rearrange("p (g r) -> p r g", g=G, r=R)

            # m = max_g |x|  -> transpose -> bf16 lhsT for the D-matmul
            nc.vector.tensor_reduce(
                m[:], xg, axis=AX.X, op=Alu.max, apply_absolute_value=True
            )
            psT1 = ps.tile([R, B], f32)
            nc.tensor.transpose(psT1[:], m[:], ident[:])
            nc.scalar.copy(mT[:], psT1[:])

            # tmp[b, (r,o)] = m[b,r] * 0.1|w[r,o]|  (block-diagonal rhs)
            tmp = ps.tile([B, 4, 4 * OC], f32)
            for k in range(4):
                nc.tensor.matmul(
                    tmp[:, k, :], mT[:], d[:, k * 4 * OC : (k + 1) * 4 * OC]
                )

            # mask-free matmul inputs on ScalarE
            nc.scalar.activation(relu1[:], x[:], Act.Relu, bias=1.0)
            nc.scalar.activation(sgn1[:], x[:], Act.Sign, bias=1.0)

            rg = relu1[:].rearrange("p (g r) -> p r g", g=G, r=R)
            sg = sgn1[:].rearrange("p (g r) -> p r g", g=G, r=R)
            nc.vector.tensor_reduce(stack3[:, 0:R], rg, axis=AX.X, op=Alu.add)
            nc.vector.tensor_reduce(stack3[:, R : 2 * R], sg, axis=AX.X, op=Alu.add)
            nc.vector.tensor_reduce(
                stack3[:, 2 * R : 3 * R],
                xg,
                axis=AX.X,
                op=Alu.add,
                apply_absolute_value=True,
            )

            nc.gpsimd.memset(stack3[:, 3 * R : 3 * R + 1], 1.0)
            psT2 = ps.tile([3 * R + 1, B], f32)
            nc.tensor.transpose(psT2[:], stack3[:], ident[:])
            nc.scalar.copy(lhsT[:], psT2[:])

            # max-part lands in PSUM, then the K=49 matmul accumulates onto it
            pout = ps.tile([B, OC], f32)
            tr = tmp[:].rearrange("p k (rr o) -> p o k rr", rr=4, o=OC)
            nc.vector.tensor_reduce(pout[:], tr, axis=AX.XY, op=Alu.max)
            nc.tensor.matmul(
                pout[:], lhsT[:], rhs[:], start=False, stop=True,
                skip_group_check=True,
            )
            out_sb = sb.tile([B, OC], f32)
            nc.scalar.copy(out_sb[:], pout[:])
            nc.sync.dma_start(out_d[:], out_sb[:])

    nc.compile()
    return nc


def _get_program():
    global _PROG
    if _PROG is None:
        _PROG = _build_program()
    return _PROG


def _host_inputs(x, weights):
    import ml_dtypes

    x = np.ascontiguousarray(np.asarray(x, dtype=np.float32))
    w = np.asarray(weights, dtype=np.float32)
    w16 = w[:R]  # (16, 1024) - only rows 0..15 are used by ROW_IDX
    in_maps = []
    for c in range(NCORES):
        wc = np.ascontiguousarray(w16[:, c * OC : (c + 1) * OC])  # (16,128)
        awc = np.abs(wc)
        d = np.zeros((R, R * OC), dtype=np.float32)
        for r in range(R):
            d[r, r * OC : (r + 1) * OC] = 0.1 * awc[r]
        rhs = np.concatenate(
            [wc, -0.5 * wc, -0.1 * awc, (-(G / 2.0) * wc.sum(axis=0))[None, :]],
            axis=0,
        ).astype(np.float32)  # (49, 128); mask count B = (B'+G)/2; bias row last
        in_maps.append(
            {
                "x": x,
                "d": d.astype(ml_dtypes.bfloat16),
                "rhs": np.ascontiguousarray(rhs),
            }
        )
    return in_maps


def kernel(x, weights):
    from concourse.bass_utils import run_bass_kernel_spmd

    nc = _get_program()
    in_maps = _host_inputs(x, weights)
    res = run_bass_kernel_spmd(nc, in_maps, core_ids=list(range(NCORES)))
    out = np.concatenate(
        [np.asarray(res.results[c]["out"]) for c in range(NCORES)], axis=1
    )
    return out.astype(np.float32)


# revision 9
# speedup vs baseline: 1.0433x; 1.0433x over previous
"""Trainium2 Bass kernel for nn_Conjunction_57793079935283.

Math: the reference expands weights via ROW_IDX = tile(arange(16), 32)
(only weight rows 0..15 are used; feature i belongs to group g = i//16
with weight row r = i%16).  The whole computation collapses to

  m[b,r] = max_g |x[b, 16g+r]|
  s[b,r] = sum_g |x[b, 16g+r]|
  A[b,r] = sum_g relu(x[b,16g+r] + 1)
  B'[b,r] = sum_g sign(x[b,16g+r] + 1)          (mask count B = (B'+32)/2)

  out = A@w16 - 0.5*B'@w16 - 16*colsum(w16)     ( == (x*mask) @ W )
        - 0.1 * (s @ |w16|)                     ( == -0.1*sum-part )
        + max_r m[b,r] * (0.1*|w16[r,o]|)       ( == +0.1*max-part )

Sharding: tensor-parallel over out_features (8 cores x 128 columns).
Per core the max-part multiply m[b,r]*0.1|w[r,o]| is done on the Tensor
engine as a matmul against a block-diagonal rhs D (host-precomputed from
weights), giving tmp[b, (r,o)] in PSUM; a single strided reduce_max over
r yields the max-part.  The matmul part runs as one K=49 matmul that
accumulates on top of the reduce_max result already sitting in PSUM.
"""

import numpy as np

_PROG = None

B = 128          # batch
G = 32           # groups per feature row
R = 16           # weight rows used (multiplicity)
OUT = 1024       # out features
NCORES = 8
OC = OUT // NCORES  # out cols per core (128)


def _build_program():
    import concourse.bacc as bacc
    import concourse.mybir as mybir
    import concourse.tile as tile

    nc = bacc.Bacc(
        "TRN2", target_bir_lowering=False, debug=False, enable_asserts=False
    )
    f32 = mybir.dt.float32
    bf16 = mybir.dt.bfloat16
    AX = mybir.AxisListType
    Alu = mybir.AluOpType
    Act = mybir.ActivationFunctionType

    x_d = nc.dram_tensor("x", [B, G * R], f32, kind="ExternalInput")
    d_d = nc.dram_tensor("d", [R, R * OC], bf16, kind="ExternalInput")
    rhs_d = nc.dram_tensor("rhs", [3 * R + 1, OC], f32, kind="ExternalInput")
    out_d = nc.dram_tensor("out", [B, OC], f32, kind="ExternalOutput")
    ident_d = nc.inline_tensor(np.eye(B, dtype=np.float32), "ident")

    with tile.TileContext(nc) as tc:
        with (
            tc.tile_pool(name="sb", bufs=1) as sb,
            tc.tile_pool(name="ps", bufs=1, space="PSUM") as ps,
        ):
            x = sb.tile([B, G * R], f32)
            d = sb.tile([R, R * OC], bf16)
            rhs = sb.tile([3 * R + 1, OC], f32)
            ident = sb.tile([B, B], f32)
            nc.sync.dma_start(x[:, 0:256], x_d[:, 0:256])
            nc.sync.dma_start(x[:, 256:512], x_d[:, 256:512])
            nc.sync.dma_start(d[:], d_d[:])
            nc.sync.dma_start(rhs[:], rhs_d[:])
            nc.sync.dma_start(ident[:], ident_d[:])

            m = sb.tile([B, R], f32)
            stack3 = sb.tile([B, 3 * R + 1], f32)
            relu1 = sb.tile([B, G * R], f32)
            sgn1 = sb.tile([B, G * R], f32)
            lhsT = sb.tile([3 * R + 1, B], f32)
            mT = sb.tile([R, B], bf16)

            xg = x[:].rearrange("p (g r) -> p r g", g=G, r=R)

            # m = max_g |x|  -> transpose -> bf16 lhsT for the D-matmul
            nc.vector.tensor_reduce(
                m[:], xg, axis=AX.X, op=Alu.max, apply_absolute_value=True
            )
            psT1 = ps.tile([R, B], f32)
            nc.tensor.transpose(psT1[:], m[:], ident[:])
            nc.scalar.copy(mT[:], psT1[:])

            # tmp[b, (r,o)] = m[b,r] * 0.1|w[r,o]|  (block-diagonal rhs)
            tmp = ps.tile([B, 4, 4 * OC], f32)
            for k in range(4):
                nc.tensor.matmul(
                    tmp[:, k, :], mT[:], d[:, k * 4 * OC : (k + 1) * 4 * OC]
                )

            # mask-free matmul inputs on ScalarE
            nc.scalar.activation(relu1[:], x[:], Act.Relu, bias=1.0)
            nc.scalar.activation(sgn1[:], x[:], Act.Sign, bias=1.0)

            rg = relu1[:].rearrange("p (g r) -> p r g", g=G, r=R)
            sg = sgn1[:].rearrange("p (g r) -> p r g", g=G, r=R)
            nc.vector.tensor_reduce(stack3[:, 0:R], rg, axis=AX.X, op=Alu.add)
            nc.vector.tensor_reduce(stack3[:, R : 2 * R], sg, axis=AX.X, op=Alu.add)
            nc.vector.tensor_reduce(
                stack3[:, 2 * R : 3 * R],
                xg,
                axis=AX.X,
                op=Alu.add,
                apply_absolute_value=True,
            )

            nc.gpsimd.memset(stack3[:, 3 * R : 3 * R + 1], 1.0)
            psT2 = ps.tile([3 * R + 1, B], f32)
            nc.tensor.transpose(psT2[:], stack3[:], ident[:])
            nc.scalar.copy(lhsT[:], psT2[:])

            # max-part via strided reduce over the block-diag matmul result
            maxp = sb.tile([B, OC], f32)
            tr = tmp[:].rearrange("p k (rr o) -> p o k rr", rr=4, o=OC)
            nc.vector.tensor_reduce(maxp[:], tr, axis=AX.XY, op=Alu.max)
            pmm = ps.tile([B, OC], f32)
            nc.tensor.matmul(pmm[:], lhsT[:], rhs[:])
            out_sb = sb.tile([B, OC], f32)
            nc.vector.tensor_add(out_sb[:], pmm[:], maxp[:])
            nc.sync.dma_start(out_d[:], out_sb[:])

    nc.compile()
    return nc


def _get_program():
    global _PROG
    if _PROG is None:
        _PROG = _build_program()
    return _PROG


def _host_inputs(x, weights):
    import ml_dtypes

    x = np.ascontiguousarray(np.asarray(x, dtype=np.float32))
    w = np.asarray(weights, dtype=np.float32)
    w16 = w[:R]  # (16, 1024) - only rows 0..15 are used by ROW_IDX
    in_maps = []
    for c in range(NCORES):
        wc = np.ascontiguousarray(w16[:, c * OC : (c + 1) * OC])  # (16,128)
        awc = np.abs(wc)
        d = np.zeros((R, R * OC), dtype=np.float32)
        for r in range(R):
            d[r, r * OC : (r + 1) * OC] = 0.1 * awc[r]
        rhs = np.concatenate(
            [wc, -0.5 * wc, -0.1 * awc, (-(G / 2.0) * wc.sum(axis=0))[None, :]],
            axis=0,
        ).astype(np.float32)  # (49, 128); mask count B = (B'+G)/2; bias row last
        in_maps.append(
            {
                "x": x,
                "d": d.astype(ml_dtypes.bfloat16),
                "rhs": np.ascontiguousarray(rhs),
            }
        )
    return in_maps


def kernel(x, weights):
    from concourse.bass_utils import run_bass_kernel_spmd

    nc = _get_program()
    in_maps = _host_inputs(x, weights)
    res = run_bass_kernel_spmd(nc, in_maps, core_ids=list(range(NCORES)))
    out = np.concatenate(
        [np.asarray(res.results[c]["out"]) for c in range(NCORES)], axis=1
    )
    return out.astype(np.float32)
